# revision 1
# baseline (speedup 1.0000x reference)
"""Trainium2 Bass kernel for the DiffSSM block.

Strategy: data-parallel over batch B=8 across the 8 NeuronCores (one batch
element per core). All heavy compute (two D x D projections, two kernel-3
convolutions over channels, and the bidirectional SSM global convolution,
expressed as a single L x L Toeplitz matmul with beta1/beta2 folded in) runs
on the TensorEngine in bf16 with fp32 PSUM accumulation. The tiny SSM kernel
generation, the timestep embedding, and the Toeplitz matrix construction are
O(L*N + B*D + L^2) host-side precomputation, replicated across cores.

Device-side dataflow per core (L=2048, D=1024, P=128):
  A: h = x @ Wi + bi (lhsT = xT), LN1 -> h_ln (L-part, D-free) bf16;
     h_ln also written to DRAM scratch.
  C: h_ln DRAM -> SBUF transposed via xbar DMA-transpose -> h_lnT (D-part,
     L-free, zero-padded borders for the conv shifts).
  B: mixT = (T_mix @ h_ln)^T via lhsT = h_ln tiles, rhs = T_mixT chunks;
     h2T = mixT * noise_scale[d] (per-partition scalar in T layout).
  D: conv1 as 3 shifted matmuls accumulated in PSUM, evicted through
     ScalarE Silu(+bc1) -> coT.
  E: conv2 likewise, fused eviction h2T += c2 + bc2.
  F: y = h2 @ Wo + bo (lhsT = h2T tiles), LN2, residual add with x,
     DMA out fp32.
"""

import math

import numpy as np
import ml_dtypes

_BF16 = ml_dtypes.bfloat16

_L, _D, _B = 2048, 1024, 8

_cache = {}


def _build(L, D, n_cores, debug_taps=False):
    import concourse.bacc as bacc
    import concourse.bass as bass
    import concourse.tile as tile
    from concourse import mybir

    f32 = mybir.dt.float32
    bf16 = mybir.dt.bfloat16
    AF = mybir.ActivationFunctionType
    OP = mybir.AluOpType

    P = 128
    KT = D // P            # feature tiles (contraction / d / o / i tiles)
    LT = L // P            # sequence tiles
    ND = min(512, D)       # matmul free-dim chunk along features
    NF = min(512, L)       # matmul free-dim chunk along sequence
    EH = D // ND
    LC = L // NF
    ST = LT

    nc = bacc.Bacc("TRN2", target_bir_lowering=False, debug=False,
                   num_devices=n_cores)

    x_res = nc.dram_tensor("x_res", (L, D), f32, kind="ExternalInput").ap()
    xT = nc.dram_tensor("xT", (D, L), bf16, kind="ExternalInput").ap()
    Wi = nc.dram_tensor("Wi", (D, D), bf16, kind="ExternalInput").ap()
    w1T = nc.dram_tensor("w1T", (KT, P, 3, D), bf16, kind="ExternalInput").ap()
    w2T = nc.dram_tensor("w2T", (KT, P, 3, D), bf16, kind="ExternalInput").ap()
    Wo = nc.dram_tensor("Wo", (D, D), bf16, kind="ExternalInput").ap()
    TmT = nc.dram_tensor("TmT", (L, L), bf16, kind="ExternalInput").ap()
    nsc = nc.dram_tensor("nsc", (P, KT), f32, kind="ExternalInput").ap()
    bc1c = nc.dram_tensor("bc1c", (P, KT), f32, kind="ExternalInput").ap()
    bc2c = nc.dram_tensor("bc2c", (P, KT), f32, kind="ExternalInput").ap()
    vec_names = ["biv", "g1v", "b1v", "g2v", "b2v", "bov"]
    vecs = {n: nc.dram_tensor(n, (D,), f32, kind="ExternalInput").ap()
            for n in vec_names}
    out = nc.dram_tensor("out", (L, D), f32, kind="ExternalOutput").ap()
    taps = {}
    if debug_taps:
        KT_ = D // 128
        taps["hln"] = nc.dram_tensor("tap_hln", (L, D), bf16,
                                     kind="ExternalOutput").ap()
        taps["hlnT"] = nc.dram_tensor("tap_hlnT", (128, KT_, L), bf16,
                                      kind="ExternalOutput").ap()
        taps["mix"] = nc.dram_tensor("tap_mix", (128, KT_, L), bf16,
                                     kind="ExternalOutput").ap()
        taps["co"] = nc.dram_tensor("tap_co", (128, KT_, L), bf16,
                                    kind="ExternalOutput").ap()
        taps["h2T"] = nc.dram_tensor("tap_h2T", (128, KT_, L), bf16,
                                     kind="ExternalOutput").ap()
        taps["wo"] = nc.dram_tensor("tap_wo", (128, KT_, D), bf16,
                                    kind="ExternalOutput").ap()
        taps["y"] = nc.dram_tensor("tap_y", (L, D), f32,
                                   kind="ExternalOutput").ap()
        taps["yln"] = nc.dram_tensor("tap_yln", (L, D), f32,
                                     kind="ExternalOutput").ap()
        taps["g2"] = nc.dram_tensor("tap_g2", (128, D), f32,
                                    kind="ExternalOutput").ap()
        taps["mv2"] = nc.dram_tensor("tap_mv2", (L // 128, 128, 2), f32,
                                     kind="ExternalOutput").ap()
        taps["fin"] = nc.dram_tensor("tap_fin", (L, D), f32,
                                     kind="ExternalOutput").ap()

    bn_fmax = math.gcd(512, D)
    n_sub = D // bn_fmax

    with tile.TileContext(nc) as tc:
        const = tc.alloc_tile_pool(name="const", bufs=1)
        psum = tc.alloc_tile_pool(name="psum", bufs=6, space="PSUM")
        statp = tc.alloc_tile_pool(name="stat", bufs=4)
        hbufp = tc.alloc_tile_pool(name="hbuf", bufs=3)
        dramp = tc.alloc_tile_pool(name="drams", bufs=1, space="DRAM")

        rep = {}
        for n in vec_names:
            t = const.tile([P, D], f32, tag=n, name=f"rep_{n}")
            ap = vecs[n]
            bcast = bass.AP(tensor=ap.tensor, offset=ap.offset,
                            ap=[[0, P]] + list(ap.ap))
            nc.gpsimd.dma_start(out=t[:], in_=bcast)
            rep[n] = t
        ns_sb = const.tile([P, KT], f32)
        nc.sync.dma_start(out=ns_sb[:], in_=nsc)
        bc1_sb = const.tile([P, KT], f32)
        nc.sync.dma_start(out=bc1_sb[:], in_=bc1c)
        bc2_sb = const.tile([P, KT], f32)
        nc.sync.dma_start(out=bc2_sb[:], in_=bc2c)
        eps_sb = const.tile([P, 1], f32)
        nc.vector.memset(eps_sb[:], 1e-5)

        h2T_pool = tc.alloc_tile_pool(name="h2T", bufs=1)
        h2T_sb = h2T_pool.tile([P, KT, L], bf16)
        hln_pool = tc.alloc_tile_pool(name="hln", bufs=1, side="right")
        hln_sb = hln_pool.tile([P, LT, D], bf16)
        hln_dram = dramp.tile([L, D], bf16)

        # ---- Phase A: proj-in + LN1 ----
        pa_pool = tc.alloc_tile_pool(name="pa", bufs=1)
        xT_sb = pa_pool.tile([P, KT, L], bf16)
        wi_sb = pa_pool.tile([P, KT, D], bf16)
        xT_r = xT.rearrange("(kt p) l -> kt p l", p=P)
        wi_r = Wi.rearrange("(kt p) d -> kt p d", p=P)
        for kt in range(KT):
            nc.sync.dma_start(out=xT_sb[:, kt, :], in_=xT_r[kt])
            nc.sync.dma_start(out=wi_sb[:, kt, :], in_=wi_r[kt])
        hd_r = hln_dram[:].rearrange("(t p) d -> t p d", p=P)

        def layer_norm(buf, g_rep, b_rep, out_ap, tap_mv=None):
            stats = statp.tile([P, n_sub, 6], f32, tag="stats", name="stats")
            for s in range(n_sub):
                nc.vector.bn_stats(out=stats[:, s, :],
                                   in_=buf[:, s * bn_fmax:(s + 1) * bn_fmax])
            mv = statp.tile([P, 2], f32, tag="mv", name="mv")
            nc.vector.bn_aggr(out=mv[:], in_=stats[:])
            if tap_mv is not None:
                nc.gpsimd.dma_start(out=tap_mv, in_=mv[:])
            rstd = statp.tile([P, 1], f32, tag="rstd", name="rstd")
            nc.scalar.activation(out=rstd[:], in_=mv[:, 1:2], func=AF.Sqrt,
                                 bias=eps_sb[:], scale=1.0)
            nc.vector.reciprocal(out=rstd[:], in_=rstd[:])
            nc.vector.tensor_scalar(out=buf[:], in0=buf[:], scalar1=mv[:, 0:1],
                                    scalar2=rstd[:], op0=OP.subtract,
                                    op1=OP.mult)
            nc.vector.tensor_mul(out=buf[:], in0=buf[:], in1=g_rep[:])
            nc.vector.tensor_add(out=out_ap, in0=buf[:], in1=b_rep[:])

        for lt in range(LT):
            h_f32 = pa_pool.tile([P, D], f32, tag="h_f32", name="h_f32",
                                 bufs=3)
            for eh in range(EH):
                ps = psum.tile([P, ND], f32, tag="ps", name="ps")
                for kt in range(KT):
                    nc.tensor.matmul(ps[:],
                                     lhsT=xT_sb[:, kt, lt * P:(lt + 1) * P],
                                     rhs=wi_sb[:, kt, eh * ND:(eh + 1) * ND],
                                     start=(kt == 0), stop=(kt == KT - 1))
                nc.vector.tensor_add(out=h_f32[:, eh * ND:(eh + 1) * ND],
                                     in0=ps[:],
                                     in1=rep["biv"][:, eh * ND:(eh + 1) * ND])
            layer_norm(h_f32, rep["g1v"], rep["b1v"], hln_sb[:, lt, :])
            nc.scalar.dma_start(out=hd_r[lt], in_=hln_sb[:, lt, :])
        pa_pool.release()
        if debug_taps:
            tap_r = taps["hln"].rearrange("(t p) d -> t p d", p=P)
            for lt in range(LT):
                nc.sync.dma_start(out=tap_r[lt], in_=hln_sb[:, lt, :])

        # ---- Phase C: transposed reload (xbar) ----
        # Xbar transpose into a fully contiguous tile at offset 0 (the only
        # destination shape validated on hardware). Conv border columns are
        # handled by narrowing the edge matmuls instead of zero padding.
        hlnT_pool = tc.alloc_tile_pool(name="hlnT", bufs=1)
        hlnT_sb = hlnT_pool.tile([P, KT, L], bf16)
        nc.scalar.dma_start_transpose(out=hlnT_sb[:], in_=hln_dram[:])

        if debug_taps:
            nc.sync.dma_start(out=taps["hlnT"], in_=hlnT_sb[:])

        # ---- Phase B: SSM Toeplitz mix ----
        tb_pool = tc.alloc_tile_pool(name="tb", bufs=2)
        Tm_r = TmT.rearrange("(st p) t -> p st t", p=P)
        for tch in range(LC):
            Tc_sb = tb_pool.tile([P, ST, NF], bf16, tag="Tc", name="Tc")
            for st in range(ST):
                nc.sync.dma_start(out=Tc_sb[:, st, :],
                                  in_=Tm_r[:, st, tch * NF:(tch + 1) * NF])
            for dt in range(KT):
                ps = psum.tile([P, NF], f32, tag="ps", name="ps")
                for st in range(ST):
                    nc.tensor.matmul(ps[:],
                                     lhsT=hln_sb[:, st, dt * P:(dt + 1) * P],
                                     rhs=Tc_sb[:, st, :],
                                     start=(st == 0), stop=(st == ST - 1))
                nc.vector.tensor_scalar_mul(
                    out=h2T_sb[:, dt, tch * NF:(tch + 1) * NF],
                    in0=ps[:], scalar1=ns_sb[:, dt:dt + 1])
        tb_pool.release()
        hln_pool.release()
        if debug_taps:
            nc.sync.dma_start(out=taps["mix"], in_=h2T_sb[:])

        # ---- Phase D: conv1 (+Silu) ----
        w1_pool = tc.alloc_tile_pool(name="w1", bufs=1)
        w1_sb = w1_pool.tile([P, KT, 3, D], bf16)
        for it in range(KT):
            nc.sync.dma_start(out=w1_sb[:, it, :, :], in_=w1T[it])
        co_pool = tc.alloc_tile_pool(name="co", bufs=1, side="right")
        co_sb = co_pool.tile([P, KT, L], bf16)

        def conv_mms(ps, w_sb, src_sb, ot, lc):
            # kernel-3 conv as 3 shifted matmuls; j=1 (no shift, full width)
            # goes first so start=True initializes the whole PSUM range, and
            # the zero-pad border columns are simply skipped.
            first = True
            for it in range(KT):
                for j in (1, 0, 2):
                    o0 = 1 if (j == 0 and lc == 0) else 0
                    o1 = NF - 1 if (j == 2 and lc == LC - 1) else NF
                    base = lc * NF + j - 1
                    nc.tensor.matmul(
                        ps[:, o0:o1],
                        lhsT=w_sb[:, it, j, ot * P:(ot + 1) * P],
                        rhs=src_sb[:, it, base + o0:base + o1],
                        start=first,
                        stop=(it == KT - 1 and j == 2))
                    first = False
        for lc in range(LC):
            for ot in range(KT):
                ps = psum.tile([P, NF], f32, tag="ps", name="ps")
                conv_mms(ps, w1_sb, hlnT_sb, ot, lc)
                nc.scalar.activation(
                    out=co_sb[:, ot, lc * NF:(lc + 1) * NF],
                    in_=ps[:], func=AF.Silu, bias=bc1_sb[:, ot:ot + 1],
                    scale=1.0)
        if debug_taps:
            nc.sync.dma_start(out=taps["co"], in_=co_sb[:])
        w1_pool.release()
        hlnT_pool.release()


        # ---- Phase E: conv2, accumulate into h2T ----
        w2_pool = tc.alloc_tile_pool(name="w2", bufs=1)
        w2_sb = w2_pool.tile([P, KT, 3, D], bf16)
        for it in range(KT):
            nc.sync.dma_start(out=w2_sb[:, it, :, :], in_=w2T[it])
        for lc in range(LC):
            for ot in range(KT):
                ps = psum.tile([P, NF], f32, tag="ps", name="ps")
                conv_mms(ps, w2_sb, co_sb, ot, lc)
                nc.vector.scalar_tensor_tensor(
                    out=h2T_sb[:, ot, lc * NF:(lc + 1) * NF],
                    in0=ps[:], scalar=bc2_sb[:, ot:ot + 1],
                    in1=h2T_sb[:, ot, lc * NF:(lc + 1) * NF],
                    op0=OP.add, op1=OP.add)
        w2_pool.release()
        co_pool.release()
        if debug_taps:
            nc.sync.dma_start(out=taps["h2T"], in_=h2T_sb[:])

        # ---- Phase F: proj-out + LN2 + residual ----
        wo_pool = tc.alloc_tile_pool(name="wo", bufs=1)
        wo_sb = wo_pool.tile([P, KT, D], bf16)
        wo_r = Wo.rearrange("(dt p) e -> dt p e", p=P)
        for dt in range(KT):
            nc.sync.dma_start(out=wo_sb[:, dt, :], in_=wo_r[dt])
        if debug_taps:
            nc.sync.dma_start(out=taps["wo"], in_=wo_sb[:])
            nc.sync.dma_start(out=taps["g2"], in_=rep["g2v"][:])
            tap_y_r = taps["y"].rearrange("(t p) d -> t p d", p=P)
            tap_yln_r = taps["yln"].rearrange("(t p) d -> t p d", p=P)
            tap_fin_r = taps["fin"].rearrange("(t p) d -> t p d", p=P)
        x_r = x_res.rearrange("(t p) d -> t p d", p=P)
        out_r = out.rearrange("(t p) d -> t p d", p=P)
        for lt in range(LT):
            x_t = hbufp.tile([P, D], f32, tag="x_t", name="x_t", bufs=2)
            nc.sync.dma_start(out=x_t[:], in_=x_r[lt])
            y = hbufp.tile([P, D], f32, tag="y", name="y", bufs=2)
            for eh in range(EH):
                ps = psum.tile([P, ND], f32, tag="ps", name="ps")
                for dt in range(KT):
                    nc.tensor.matmul(ps[:],
                                     lhsT=h2T_sb[:, dt, lt * P:(lt + 1) * P],
                                     rhs=wo_sb[:, dt, eh * ND:(eh + 1) * ND],
                                     start=(dt == 0), stop=(dt == KT - 1))
                nc.vector.tensor_add(out=y[:, eh * ND:(eh + 1) * ND],
                                     in0=ps[:],
                                     in1=rep["bov"][:, eh * ND:(eh + 1) * ND])
            if debug_taps:
                nc.sync.dma_start(out=tap_y_r[lt], in_=y[:])
            layer_norm(y, rep["g2v"], rep["b2v"], y[:],
                       tap_mv=(taps["mv2"][lt] if debug_taps else None))
            if debug_taps:
                nc.sync.dma_start(out=tap_yln_r[lt], in_=y[:])
            out_t = hbufp.tile([P, D], f32, tag="out_t", name="out_t",
                               bufs=2)
            nc.vector.tensor_add(out=out_t[:], in0=y[:], in1=x_t[:])
            nc.sync.dma_start(out=out_r[lt], in_=out_t[:])
            if debug_taps:
                nc.sync.dma_start(out=tap_fin_r[lt], in_=out_t[:])
        wo_pool.release()
        h2T_pool.release()
        dramp.release()
        hbufp.release()
        statp.release()
        psum.release()
        const.release()

    nc.compile()
    return nc


def _bf(a):
    return np.ascontiguousarray(np.asarray(a, np.float32)).astype(_BF16)


def _prep_maps(inputs, L, D, n_cores):
    P = 128
    KT = D // P
    f32 = np.float32
    x = np.asarray(inputs["x"], f32)
    t = np.asarray(inputs["t"], f32)
    beta1 = float(np.asarray(inputs["beta1"], f32)[0])
    beta2 = float(np.asarray(inputs["beta2"], f32)[0])

    # SSM kernels -> mixed Toeplitz (transposed), host fp32
    af = np.diagonal(np.asarray(inputs["Af"], f32))
    ab = np.diagonal(np.asarray(inputs["Ab"], f32))
    l_ar = np.arange(L, dtype=f32)[:, None]
    kf = np.exp(l_ar * af[None, :]) @ (
        np.asarray(inputs["Bf"], f32)[:, 0] * np.asarray(inputs["Cf"], f32)[0]
    ) + np.asarray(inputs["Df"], f32)[0]
    kb = np.exp(l_ar * ab[None, :]) @ (
        np.asarray(inputs["Bb"], f32)[:, 0] * np.asarray(inputs["Cb"], f32)[0]
    ) + np.asarray(inputs["Db"], f32)[0]
    tms = np.arange(L)[None, :] - np.arange(L)[:, None]   # T_mixT[s,t] : t-s
    TmT = (np.where(tms >= 0, beta1 * kf[np.clip(tms, 0, None)], 0.0)
           + np.where(tms <= 0, beta2 * kb[np.clip(-tms, 0, None)], 0.0))
    TmT_bf = TmT.astype(f32).astype(_BF16)

    # timestep embedding -> noise scale (B, D)
    half = D // 2
    freqs = np.exp(np.arange(half, dtype=f32)
                   * (-math.log(10000.0) / (half - 1)))
    ang = t[:, None] * freqs[None, :]
    emb = np.concatenate([np.sin(ang), np.cos(ang)], axis=1).astype(f32)
    ns = (1.0 / (1.0 + np.exp(-emb))).astype(f32)         # (B, D)

    Wi_bf = _bf(inputs["Wi"])
    Wo_bf = _bf(inputs["Wo"])
    w1 = np.asarray(inputs["w1"], f32)
    w2 = np.asarray(inputs["w2"], f32)
    w1T = np.ascontiguousarray(np.transpose(w1, (1, 2, 0))).reshape(
        KT, P, 3, D).astype(_BF16)
    w2T = np.ascontiguousarray(np.transpose(w2, (1, 2, 0))).reshape(
        KT, P, 3, D).astype(_BF16)

    def col(v):
        return np.ascontiguousarray(
            np.asarray(v, f32).reshape(KT, P).T)

    shared = {
        "Wi": Wi_bf, "Wo": Wo_bf, "w1T": w1T, "w2T": w2T, "TmT": TmT_bf,
        "bc1c": col(inputs["bc1"]), "bc2c": col(inputs["bc2"]),
        "biv": np.ascontiguousarray(np.asarray(inputs["bi"], f32)),
        "g1v": np.ascontiguousarray(np.asarray(inputs["g1"], f32)),
        "b1v": np.ascontiguousarray(np.asarray(inputs["b1"], f32)),
        "g2v": np.ascontiguousarray(np.asarray(inputs["g2"], f32)),
        "b2v": np.ascontiguousarray(np.asarray(inputs["b2"], f32)),
        "bov": np.ascontiguousarray(np.asarray(inputs["bo"], f32)),
    }
    in_maps = []
    for b in range(n_cores):
        xb = np.ascontiguousarray(x[b])
        m = dict(shared)
        m["x_res"] = xb
        m["xT"] = np.ascontiguousarray(xb.T.astype(_BF16))
        m["nsc"] = np.ascontiguousarray(ns[b].reshape(KT, P).T)
        in_maps.append(m)
    return in_maps


def get_nc(L=_L, D=_D, n_cores=_B, debug_taps=False):
    key = (L, D, n_cores, debug_taps)
    if key not in _cache:
        _cache[key] = _build(L, D, n_cores, debug_taps)
    return _cache[key]


def kernel(**inputs):
    from concourse.bass_utils import run_bass_kernel_spmd

    L, D, B = _L, _D, _B
    nc = get_nc(L, D, B)
    in_maps = _prep_maps(inputs, L, D, B)
    res = run_bass_kernel_spmd(nc, in_maps, core_ids=list(range(B)))
    return np.stack([res.results[c]["out"] for c in range(B)]).astype(
        np.float32)

